# revision 8
# baseline (speedup 1.0000x reference)
"""Multi-head attention (B=2, S=2048, d_model=1024, 16 heads) on 8 trn2 cores.

Sharding: core c -> batch c//4, head-group c%4 (4 heads, 256 feature cols).
Per-core kernel, for its batch's tokens x and its 256-col slices of Wq/Wk/Wv
(col-parallel) and 256-row slice of Wo (row-parallel):
  QT/KT = (x @ W + b).T   [256, 2048]  f32r (tf32 matmuls, fp32 accumulate)
  V     = x @ Wv + bv     [2048, 256]  bf16 seq-major
  per head: scores = Q K^T (f32r, K=64) -> exp(x/8) on ACT, bf16, with fused
  row-sum (no max-sub; scores are O(5)) -> attn = exp * (1/rsum) in-place
  (DVE bf16 2x) -> DMA'd out + XBAR DMA-transposed into k-major attT ->
  ctxT = V^T @ attnT (bf16) -> out_part = ctx @ Wo (f32r, f32 out;
  bias bo added on host).
Host sums the 4 per-batch out_part partials and concatenates attn heads.
"""

import os
import sys

for _p in ("/opt/trn_rl_repo", "/opt/pypackages"):
    if os.path.isdir(_p) and _p not in sys.path:
        sys.path.insert(0, _p)

import numpy as np

import concourse.bass as bass
import concourse.tile as tile
from concourse import bacc, mybir
from concourse.bass_utils import run_bass_kernel_spmd

P = 128
S = 2048
D_MODEL = 1024
NHEAD = 16
DK = 64
HPC = 4            # heads per core
DPC = HPC * DK     # 256: feature columns per core
KT_TILES = D_MODEL // P   # 8 k-tiles over the contraction dim
ST_TILES = S // P         # 16 tiles over the sequence
F32 = mybir.dt.float32
F32R = mybir.dt.float32r
BF16 = mybir.dt.bfloat16
AF = mybir.ActivationFunctionType


def _build_nc():
    nc = bacc.Bacc("TRN2", target_bir_lowering=False, debug=False)

    xqT_d = nc.dram_tensor("xqT", [D_MODEL, S], F32R, kind="ExternalInput").ap()
    xkT_d = nc.dram_tensor("xkT", [D_MODEL, S], F32R, kind="ExternalInput").ap()
    xvT_d = nc.dram_tensor("xvT", [D_MODEL, S], F32R, kind="ExternalInput").ap()
    wq_d = nc.dram_tensor("wq", [D_MODEL, DPC], F32R, kind="ExternalInput").ap()
    wk_d = nc.dram_tensor("wk", [D_MODEL, DPC], F32R, kind="ExternalInput").ap()
    wv_d = nc.dram_tensor("wv", [D_MODEL, DPC], F32R, kind="ExternalInput").ap()
    wo_d = nc.dram_tensor("wo", [DPC, D_MODEL], F32R, kind="ExternalInput").ap()
    bq_d = nc.dram_tensor("bq", [1, DPC], F32R, kind="ExternalInput").ap()
    bk_d = nc.dram_tensor("bk", [1, DPC], F32R, kind="ExternalInput").ap()
    bv_d = nc.dram_tensor("bv", [1, DPC], F32R, kind="ExternalInput").ap()
    attn_d = nc.dram_tensor("attn_o", [HPC, S, S], BF16, kind="ExternalOutput").ap()
    out_d = nc.dram_tensor("out_o", [S, D_MODEL], F32, kind="ExternalOutput").ap()

    with tile.TileContext(nc) as tc:
        with (
            tc.tile_pool(name="cpool", bufs=1) as cpool,
            tc.tile_pool(name="xpool", bufs=10) as xpool,
            tc.tile_pool(name="qkv", bufs=1) as qkv,
            tc.tile_pool(name="work", bufs=3) as work,
            tc.tile_pool(name="atp", bufs=2) as atp,
            tc.tile_pool(name="stats", bufs=8) as stats,
            tc.tile_pool(name="ps_big", bufs=3, space="PSUM") as ps_big,
            tc.tile_pool(name="ps_acc", bufs=2, space="PSUM") as ps_acc,
        ):
            ones = cpool.tile([1, 512], F32R)
            nc.gpsimd.memset(ones.bitcast(mybir.dt.uint32), 0x3F800000)
            zbias = cpool.tile([P, 1], F32)
            nc.gpsimd.memset(zbias, 0.0)

            wq_sb = cpool.tile([P, KT_TILES, DPC], F32R)
            nc.sync.dma_start(wq_sb, wq_d.rearrange("(kt p) m -> p kt m", p=P))
            wk_sb = cpool.tile([P, KT_TILES, DPC], F32R)
            nc.sync.dma_start(wk_sb, wk_d.rearrange("(kt p) m -> p kt m", p=P))
            wv_sb = cpool.tile([P, KT_TILES, DPC], F32R)
            nc.sync.dma_start(wv_sb, wv_d.rearrange("(kt p) m -> p kt m", p=P))
            wo_sb = cpool.tile([P, 2, D_MODEL], F32R)
            nc.sync.dma_start(wo_sb, wo_d.rearrange("(kt p) m -> p kt m", p=P))
            bq_sb = cpool.tile([1, DPC], F32R)
            nc.sync.dma_start(bq_sb, bq_d)
            bk_sb = cpool.tile([1, DPC], F32R)
            nc.sync.dma_start(bk_sb, bk_d)
            bv_sb = cpool.tile([1, DPC], F32R)
            nc.sync.dma_start(bv_sb, bv_d)

            QT = qkv.tile([P, 2, S], F32R)   # [d%128, d//128, q]
            KT = qkv.tile([P, 2, S], F32R)
            Vsb = qkv.tile([P, ST_TILES, DPC], BF16)  # [s%128, s//128, d]
            ctxT = qkv.tile([P, 2, S], F32R)

            # ---- Q/K projections, feature-major: dst[d, q] = W.T @ xT + b ----
            # x streamed as [128, 1024] half-tiles; nn-outer keeps 8 live
            for xdram, w_sb, b_sb, dst in (
                (xqT_d, wq_sb, bq_sb, QT),
                (xkT_d, wk_sb, bk_sb, KT),
            ):
                for nn in range(2):
                    xh = []
                    for kt in range(KT_TILES):
                        xt_t = xpool.tile([P, 1024], F32R, tag="b4k", name=f"xh{kt}")
                        nc.sync.dma_start(
                            xt_t,
                            xdram[kt * P : (kt + 1) * P, nn * 1024 : (nn + 1) * 1024],
                        )
                        xh.append(xt_t)
                    for m in range(2):
                        ps = ps_big.tile([P, 1024], F32, tag="big", name="proj_ps")
                        for kt in range(KT_TILES):
                            for sub in range(2):
                                nc.tensor.matmul(
                                    ps[:, sub * 512 : (sub + 1) * 512],
                                    w_sb[:, kt, m * P : (m + 1) * P],
                                    xh[kt][:, sub * 512 : (sub + 1) * 512],
                                    start=(kt == 0),
                                    stop=False,
                                )
                        for sub in range(2):
                            nc.tensor.matmul(
                                ps[:, sub * 512 : (sub + 1) * 512],
                                b_sb[:, m * P : (m + 1) * P],
                                ones,
                                start=False,
                                stop=True,
                            )
                        nc.any.tensor_copy(
                            dst[:, m, nn * 1024 : (nn + 1) * 1024], ps
                        )

            # ---- V projection, seq-major bf16: V[s, d] = xT.T @ Wv + bv ----
            xv = {}
            for nn in range(2):
                for kt in range(KT_TILES):
                    xt_t = xpool.tile([P, 1024], F32R, tag="b4k", name=f"xv{kt}")
                    nc.sync.dma_start(
                        xt_t,
                        xvT_d[kt * P : (kt + 1) * P, nn * 1024 : (nn + 1) * 1024],
                    )
                    xv[nn, kt] = xt_t
                for st8 in range(8):
                    st = nn * 8 + st8
                    ps = ps_big.tile([P, 1024], F32, tag="big", name="v_ps")
                    psv = ps[:, :DPC]
                    for kt in range(KT_TILES):
                        nc.tensor.matmul(
                            psv,
                            xv[nn, kt][:, st8 * P : (st8 + 1) * P],
                            wv_sb[:, kt, :],
                            start=(kt == 0),
                            stop=False,
                        )
                    nc.tensor.matmul(
                        psv, ones[:, :P], bv_sb, start=False, stop=True
                    )
                    nc.any.tensor_copy(Vsb[:, st, :], psv)

            # ---- attention per head ----
            for h in range(HPC):
                ti, pr = h // 2, (h % 2) * DK
                attT = None
                for qt in range(ST_TILES):
                    at = work.tile([P, S], BF16, tag="at", name="at")
                    rs2 = stats.tile([P, 2], F32, tag="rs2", name="rs2")
                    for half in range(2):
                        ps = ps_big.tile([P, 1024], F32, tag="big", name="sc_ps")
                        for sub in range(2):
                            col = half * 1024 + sub * 512
                            nc.tensor.matmul(
                                ps[:, sub * 512 : (sub + 1) * 512],
                                QT[pr : pr + DK, ti, qt * P : (qt + 1) * P],
                                KT[pr : pr + DK, ti, col : col + 512],
                                start=True,
                                stop=True,
                            )
                        nc.scalar.activation(
                            at[:, half * 1024 : (half + 1) * 1024],
                            ps,
                            AF.Exp,
                            bias=zbias,
                            scale=0.125,
                            accum_out=rs2[:, half : half + 1],
                        )
                    rsum = stats.tile([P, 1], F32, tag="rs1", name="rsum")
                    nc.vector.reduce_sum(rsum, rs2, axis=mybir.AxisListType.X)
                    inv = stats.tile([P, 1], F32, tag="inv", name="inv")
                    nc.vector.reciprocal(inv, rsum)
                    nc.vector.tensor_scalar_mul(at, at, inv)
                    nc.sync.dma_start(attn_d[h, qt * P : (qt + 1) * P, :], at)

                    # XBAR transpose the normalized row-block into k-major attT
                    if qt % 4 == 0:
                        attT = atp.tile([P, ST_TILES, 512], BF16, tag="attT",
                                        name="attT")
                    c0 = (qt % 4) * P
                    nc.scalar.dma_start_transpose(attT[:, :, c0 : c0 + P], at)

                    if qt % 4 == 3:
                        qc = qt // 4
                        cps = ps_acc.tile([DK, 512], F32, tag="acc", name="cps")
                        for j in range(ST_TILES):
                            nc.tensor.matmul(
                                cps,
                                Vsb[:, j, h * DK : (h + 1) * DK],
                                attT[:, j, :],
                                start=(j == 0),
                                stop=(j == ST_TILES - 1),
                            )
                        nc.any.tensor_copy(
                            ctxT[pr : pr + DK, ti, qc * 512 : (qc + 1) * 512], cps
                        )

            # ---- output projection: out[q, :] = ctx @ Wo ----
            for qt in range(ST_TILES):
                ob = work.tile([P, D_MODEL], F32, tag="ob", name="ob", bufs=2)
                pss = [
                    ps_acc.tile([P, 512], F32, tag="acc", name="o_ps")
                    for _ in range(2)
                ]
                for kt2 in range(2):
                    for nco in range(2):
                        nc.tensor.matmul(
                            pss[nco],
                            ctxT[:, kt2, qt * P : (qt + 1) * P],
                            wo_sb[:, kt2, nco * 512 : (nco + 1) * 512],
                            start=(kt2 == 0),
                            stop=(kt2 == 1),
                        )
                for nco in range(2):
                    nc.any.tensor_copy(ob[:, nco * 512 : (nco + 1) * 512], pss[nco])
                nc.sync.dma_start(out_d[qt * P : (qt + 1) * P, :], ob)

    nc.compile()
    return nc


_NC_CACHE = {}


def _get_nc():
    if "nc" not in _NC_CACHE:
        _NC_CACHE["nc"] = _build_nc()
    return _NC_CACHE["nc"]


def make_in_maps(query, key, value, Wq, bq, Wk, bk, Wv, bv, Wo, bo=None):
    query = np.asarray(query, np.float32)
    key = np.asarray(key, np.float32)
    value = np.asarray(value, np.float32)
    Wq = np.asarray(Wq, np.float32)
    Wk = np.asarray(Wk, np.float32)
    Wv = np.asarray(Wv, np.float32)
    Wo = np.asarray(Wo, np.float32)
    bq = np.asarray(bq, np.float32)
    bk = np.asarray(bk, np.float32)
    bv = np.asarray(bv, np.float32)

    xT = {}
    for b in range(2):
        xT[b] = (
            np.ascontiguousarray(query[b].T),
            np.ascontiguousarray(key[b].T),
            np.ascontiguousarray(value[b].T),
        )
    in_maps = []
    for c in range(8):
        b, g = divmod(c, 4)
        ds = slice(g * DPC, (g + 1) * DPC)
        xq, xk, xv = xT[b]
        in_maps.append(
            {
                "xqT": xq,
                "xkT": xk,
                "xvT": xv,
                "wq": np.ascontiguousarray(Wq[:, ds]),
                "wk": np.ascontiguousarray(Wk[:, ds]),
                "wv": np.ascontiguousarray(Wv[:, ds]),
                "wo": np.ascontiguousarray(Wo[ds, :]),
                "bq": np.ascontiguousarray(bq[ds]).reshape(1, DPC),
                "bk": np.ascontiguousarray(bk[ds]).reshape(1, DPC),
                "bv": np.ascontiguousarray(bv[ds]).reshape(1, DPC),
            }
        )
    return in_maps


def assemble(results, bo):
    bo = np.asarray(bo, np.float32)
    attn = np.empty((2, NHEAD, S, S), np.float32)
    out = np.zeros((2, S, D_MODEL), np.float32)
    for c in range(8):
        b, g = divmod(c, 4)
        attn[b, g * HPC : (g + 1) * HPC] = np.asarray(
            results[c]["attn_o"], np.float32
        )
        out[b] += results[c]["out_o"]
    out += bo
    return out, attn


def run(trace=False, trace_cores=None, **inputs):
    nc = _get_nc()
    in_maps = make_in_maps(**{k: v for k, v in inputs.items() if k != "bo"})
    res = run_bass_kernel_spmd(
        nc,
        in_maps,
        core_ids=list(range(8)),
        trace=trace,
        trace_cores=trace_cores,
    )
    out, attn = assemble(res.results, inputs["bo"])
    return out, attn, res


def kernel(query, key, value, Wq, bq, Wk, bk, Wv, bv, Wo, bo):
    out, attn, _ = run(
        query=query, key=key, value=value, Wq=Wq, bq=bq, Wk=Wk, bk=bk,
        Wv=Wv, bv=bv, Wo=Wo, bo=bo,
    )
    return out, attn


# revision 9
# speedup vs baseline: 1.4046x; 1.4046x over previous
"""Multi-head attention (B=2, S=2048, d_model=1024, 16 heads) on 8 trn2 cores.

Sharding: core c -> batch c//4, head-group c%4 (4 heads, 256 feature cols).
Per-core kernel, for its batch's tokens x and its 256-col slices of Wq/Wk/Wv
(col-parallel) and 256-row slice of Wo (row-parallel):
  QT/KT = (x @ W + b).T   [256, 2048]  f32r (tf32 matmuls, fp32 accumulate)
  V     = x @ Wv + bv     [2048, 256]  bf16 seq-major
  per head: scores = Q K^T (f32r, K=64) -> exp(x/8) on ACT, bf16, with fused
  row-sum (no max-sub; scores are O(5)) -> attn = exp * (1/rsum) in-place
  (DVE bf16 2x) -> DMA'd out + XBAR DMA-transposed into k-major attT ->
  ctxT = V^T @ attnT (bf16) -> out_part = ctx @ Wo (f32r, f32 out;
  bias bo added on host).
Host sums the 4 per-batch out_part partials and concatenates attn heads.
"""

import os
import sys

for _p in ("/opt/trn_rl_repo", "/opt/pypackages"):
    if os.path.isdir(_p) and _p not in sys.path:
        sys.path.insert(0, _p)

import numpy as np

import concourse.bass as bass
import concourse.tile as tile
from concourse import bacc, mybir
from concourse.bass_utils import run_bass_kernel_spmd

P = 128
S = 2048
D_MODEL = 1024
NHEAD = 16
DK = 64
HPC = 4            # heads per core
DPC = HPC * DK     # 256: feature columns per core
KT_TILES = D_MODEL // P   # 8 k-tiles over the contraction dim
ST_TILES = S // P         # 16 tiles over the sequence
F32 = mybir.dt.float32
F32R = mybir.dt.float32r
BF16 = mybir.dt.bfloat16
AF = mybir.ActivationFunctionType


def _build_nc():
    nc = bacc.Bacc("TRN2", target_bir_lowering=False, debug=False)

    xqT_d = nc.dram_tensor("xqT", [D_MODEL, S], F32R, kind="ExternalInput").ap()
    xkT_d = nc.dram_tensor("xkT", [D_MODEL, S], F32R, kind="ExternalInput").ap()
    xvT_d = nc.dram_tensor("xvT", [D_MODEL, S], F32R, kind="ExternalInput").ap()
    wq_d = nc.dram_tensor("wq", [D_MODEL, DPC], F32R, kind="ExternalInput").ap()
    wk_d = nc.dram_tensor("wk", [D_MODEL, DPC], F32R, kind="ExternalInput").ap()
    wv_d = nc.dram_tensor("wv", [D_MODEL, DPC], F32R, kind="ExternalInput").ap()
    wo_d = nc.dram_tensor("wo", [DPC, D_MODEL], F32R, kind="ExternalInput").ap()
    bq_d = nc.dram_tensor("bq", [1, DPC], F32R, kind="ExternalInput").ap()
    bk_d = nc.dram_tensor("bk", [1, DPC], F32R, kind="ExternalInput").ap()
    bv_d = nc.dram_tensor("bv", [1, DPC], F32R, kind="ExternalInput").ap()
    attn_d = nc.dram_tensor("attn_o", [HPC, S, S], BF16, kind="ExternalOutput").ap()
    out_d = nc.dram_tensor("out_o", [S, D_MODEL], F32, kind="ExternalOutput").ap()

    with tile.TileContext(nc) as tc:
        with (
            tc.tile_pool(name="cpool", bufs=1) as cpool,
            tc.tile_pool(name="xpool", bufs=9) as xpool,
            tc.tile_pool(name="qkv", bufs=1) as qkv,
            tc.tile_pool(name="work", bufs=3) as work,
            tc.tile_pool(name="atp", bufs=1) as atp,
            tc.tile_pool(name="stats", bufs=8) as stats,
            tc.tile_pool(name="ps_big", bufs=3, space="PSUM") as ps_big,
            tc.tile_pool(name="ps_acc", bufs=2, space="PSUM") as ps_acc,
        ):
            ones = cpool.tile([1, 512], F32R)
            nc.gpsimd.memset(ones.bitcast(mybir.dt.uint32), 0x3F800000)
            zbias = cpool.tile([P, 1], F32)
            nc.gpsimd.memset(zbias, 0.0)

            wq_sb = cpool.tile([P, KT_TILES, DPC], F32R)
            nc.sync.dma_start(wq_sb, wq_d.rearrange("(kt p) m -> p kt m", p=P))
            wk_sb = cpool.tile([P, KT_TILES, DPC], F32R)
            nc.sync.dma_start(wk_sb, wk_d.rearrange("(kt p) m -> p kt m", p=P))
            wv_sb = cpool.tile([P, KT_TILES, DPC], F32R)
            nc.sync.dma_start(wv_sb, wv_d.rearrange("(kt p) m -> p kt m", p=P))
            wo_sb = cpool.tile([P, 2, D_MODEL], F32R)
            nc.sync.dma_start(wo_sb, wo_d.rearrange("(kt p) m -> p kt m", p=P))
            bq_sb = cpool.tile([1, DPC], F32R)
            nc.sync.dma_start(bq_sb, bq_d)
            bk_sb = cpool.tile([1, DPC], F32R)
            nc.sync.dma_start(bk_sb, bk_d)
            bv_sb = cpool.tile([1, DPC], F32R)
            nc.sync.dma_start(bv_sb, bv_d)

            QT = qkv.tile([P, 2, S], F32R)   # [d%128, d//128, q]
            KT = qkv.tile([P, 2, S], F32R)
            Vsb = qkv.tile([P, ST_TILES, DPC], BF16)  # [s%128, s//128, d]
            ctxT = qkv.tile([P, 2, S], F32R)

            # ---- Q/K projections, feature-major: dst[d, q] = W.T @ xT + b ----
            # x streamed as [128, 1024] half-tiles; nn-outer keeps 8 live
            for xdram, w_sb, b_sb, dst in (
                (xqT_d, wq_sb, bq_sb, QT),
                (xkT_d, wk_sb, bk_sb, KT),
            ):
                for nn in range(2):
                    xh = []
                    for kt in range(KT_TILES):
                        xt_t = xpool.tile([P, 1024], F32R, tag="b4k", name=f"xh{kt}")
                        nc.sync.dma_start(
                            xt_t,
                            xdram[kt * P : (kt + 1) * P, nn * 1024 : (nn + 1) * 1024],
                        )
                        xh.append(xt_t)
                    for m in range(2):
                        ps = ps_big.tile([P, 1024], F32, tag="big", name="proj_ps")
                        for kt in range(KT_TILES):
                            for sub in range(2):
                                nc.tensor.matmul(
                                    ps[:, sub * 512 : (sub + 1) * 512],
                                    w_sb[:, kt, m * P : (m + 1) * P],
                                    xh[kt][:, sub * 512 : (sub + 1) * 512],
                                    start=(kt == 0),
                                    stop=False,
                                )
                        for sub in range(2):
                            nc.tensor.matmul(
                                ps[:, sub * 512 : (sub + 1) * 512],
                                b_sb[:, m * P : (m + 1) * P],
                                ones,
                                start=False,
                                stop=True,
                            )
                        nc.any.tensor_copy(
                            dst[:, m, nn * 1024 : (nn + 1) * 1024], ps
                        )

            # ---- V projection, seq-major bf16: V[s, d] = xT.T @ Wv + bv ----
            xv = {}
            for nn in range(2):
                for kt in range(KT_TILES):
                    xt_t = xpool.tile([P, 1024], F32R, tag="b4k", name=f"xv{kt}")
                    nc.sync.dma_start(
                        xt_t,
                        xvT_d[kt * P : (kt + 1) * P, nn * 1024 : (nn + 1) * 1024],
                    )
                    xv[nn, kt] = xt_t
                for st8 in range(8):
                    st = nn * 8 + st8
                    ps = ps_big.tile([P, 1024], F32, tag="big", name="v_ps")
                    psv = ps[:, :DPC]
                    for kt in range(KT_TILES):
                        nc.tensor.matmul(
                            psv,
                            xv[nn, kt][:, st8 * P : (st8 + 1) * P],
                            wv_sb[:, kt, :],
                            start=(kt == 0),
                            stop=False,
                        )
                    nc.tensor.matmul(
                        psv, ones[:, :P], bv_sb, start=False, stop=True
                    )
                    nc.any.tensor_copy(Vsb[:, st, :], psv)

            # ---- attention per head ----
            # at_wide holds a 4-q-tile chunk [128, 4*2048]; one XBAR
            # transpose + one attn store per chunk.
            for h in range(HPC):
                ti, pr = h // 2, (h % 2) * DK
                for qc in range(4):
                    atw = work.tile([P, 4 * S], BF16, tag="atw", name="atw")
                    invs = []
                    for ql in range(4):
                        qt = qc * 4 + ql
                        rs2 = stats.tile([P, 2], F32, tag="rs2", name="rs2")
                        for half in range(2):
                            ps = ps_big.tile([P, 1024], F32, tag="big", name="sc_ps")
                            for sub in range(2):
                                col = half * 1024 + sub * 512
                                nc.tensor.matmul(
                                    ps[:, sub * 512 : (sub + 1) * 512],
                                    QT[pr : pr + DK, ti, qt * P : (qt + 1) * P],
                                    KT[pr : pr + DK, ti, col : col + 512],
                                    start=True,
                                    stop=True,
                                )
                            nc.scalar.activation(
                                atw[:, ql * S + half * 1024 : ql * S + (half + 1) * 1024],
                                ps,
                                AF.Exp,
                                bias=zbias,
                                scale=0.125,
                                accum_out=rs2[:, half : half + 1],
                            )
                        rsum = stats.tile([P, 1], F32, tag="rs1", name="rsum")
                        nc.vector.reduce_sum(rsum, rs2, axis=mybir.AxisListType.X)
                        inv = stats.tile([P, 1], F32, tag="inv", name="inv")
                        nc.vector.reciprocal(inv, rsum)
                        nc.vector.tensor_scalar_mul(
                            atw[:, ql * S : (ql + 1) * S],
                            atw[:, ql * S : (ql + 1) * S],
                            inv,
                        )
                        invs.append(inv)
                    # one store + one k-major transpose for the whole chunk
                    nc.sync.dma_start(
                        attn_d[h, qc * 512 : (qc + 1) * 512, :].rearrange(
                            "(ql p) k -> p ql k", p=P
                        ),
                        atw.rearrange("p (ql k) -> p ql k", ql=4),
                    )
                    attT = atp.tile([P, 64, P], BF16, tag="attT", name="attT")
                    nc.sync.dma_start_transpose(attT, atw)
                    attv = attT.rearrange("p (ql kt) q -> p kt ql q", kt=ST_TILES)
                    cps = ps_acc.tile([DK, 512], F32, tag="acc", name="cps")
                    for j in range(ST_TILES):
                        nc.tensor.matmul(
                            cps,
                            Vsb[:, j, h * DK : (h + 1) * DK],
                            attv[:, j],
                            start=(j == 0),
                            stop=(j == ST_TILES - 1),
                        )
                    nc.any.tensor_copy(
                        ctxT[pr : pr + DK, ti, qc * 512 : (qc + 1) * 512], cps
                    )

            # ---- output projection: out[q, :] = ctx @ Wo ----
            for qt in range(ST_TILES):
                ob = work.tile([P, D_MODEL], F32, tag="ob", name="ob", bufs=2)
                pss = [
                    ps_acc.tile([P, 512], F32, tag="acc", name="o_ps")
                    for _ in range(2)
                ]
                for kt2 in range(2):
                    for nco in range(2):
                        nc.tensor.matmul(
                            pss[nco],
                            ctxT[:, kt2, qt * P : (qt + 1) * P],
                            wo_sb[:, kt2, nco * 512 : (nco + 1) * 512],
                            start=(kt2 == 0),
                            stop=(kt2 == 1),
                        )
                for nco in range(2):
                    nc.any.tensor_copy(ob[:, nco * 512 : (nco + 1) * 512], pss[nco])
                nc.sync.dma_start(out_d[qt * P : (qt + 1) * P, :], ob)

    nc.compile()
    return nc


_NC_CACHE = {}


def _get_nc():
    if "nc" not in _NC_CACHE:
        _NC_CACHE["nc"] = _build_nc()
    return _NC_CACHE["nc"]


def make_in_maps(query, key, value, Wq, bq, Wk, bk, Wv, bv, Wo, bo=None):
    query = np.asarray(query, np.float32)
    key = np.asarray(key, np.float32)
    value = np.asarray(value, np.float32)
    Wq = np.asarray(Wq, np.float32)
    Wk = np.asarray(Wk, np.float32)
    Wv = np.asarray(Wv, np.float32)
    Wo = np.asarray(Wo, np.float32)
    bq = np.asarray(bq, np.float32)
    bk = np.asarray(bk, np.float32)
    bv = np.asarray(bv, np.float32)

    xT = {}
    for b in range(2):
        xT[b] = (
            np.ascontiguousarray(query[b].T),
            np.ascontiguousarray(key[b].T),
            np.ascontiguousarray(value[b].T),
        )
    in_maps = []
    for c in range(8):
        b, g = divmod(c, 4)
        ds = slice(g * DPC, (g + 1) * DPC)
        xq, xk, xv = xT[b]
        in_maps.append(
            {
                "xqT": xq,
                "xkT": xk,
                "xvT": xv,
                "wq": np.ascontiguousarray(Wq[:, ds]),
                "wk": np.ascontiguousarray(Wk[:, ds]),
                "wv": np.ascontiguousarray(Wv[:, ds]),
                "wo": np.ascontiguousarray(Wo[ds, :]),
                "bq": np.ascontiguousarray(bq[ds]).reshape(1, DPC),
                "bk": np.ascontiguousarray(bk[ds]).reshape(1, DPC),
                "bv": np.ascontiguousarray(bv[ds]).reshape(1, DPC),
            }
        )
    return in_maps


def assemble(results, bo):
    bo = np.asarray(bo, np.float32)
    attn = np.empty((2, NHEAD, S, S), np.float32)
    out = np.zeros((2, S, D_MODEL), np.float32)
    for c in range(8):
        b, g = divmod(c, 4)
        attn[b, g * HPC : (g + 1) * HPC] = np.asarray(
            results[c]["attn_o"], np.float32
        )
        out[b] += results[c]["out_o"]
    out += bo
    return out, attn


def run(trace=False, trace_cores=None, **inputs):
    nc = _get_nc()
    in_maps = make_in_maps(**{k: v for k, v in inputs.items() if k != "bo"})
    res = run_bass_kernel_spmd(
        nc,
        in_maps,
        core_ids=list(range(8)),
        trace=trace,
        trace_cores=trace_cores,
    )
    out, attn = assemble(res.results, inputs["bo"])
    return out, attn, res


def kernel(query, key, value, Wq, bq, Wk, bk, Wv, bv, Wo, bo):
    out, attn, _ = run(
        query=query, key=key, value=value, Wq=Wq, bq=bq, Wk=Wk, bk=bk,
        Wv=Wv, bv=bv, Wo=Wo, bo=bo,
    )
    return out, attn
